# revision 13
# baseline (speedup 1.0000x reference)
"""NOTEARS loss kernel for Trainium2 (8 NeuronCores, Bass/Tile).

Math: with W_m = W with zeroed diagonal, A = I - W_m^T, G = X^T X:
    ||X - X W_m^T||_F^2 = tr(A^T G A)
so the only T-sized work is the Gram accumulation G = X^T X.  X is
sharded row-wise over 8 cores (data parallel) and quantized to fp8
(e3m4) on the host — the quantization perturbs the loss by ~3e-4
relative (tolerance 2e-2) and both halves DMA traffic and runs the
PE at 1 cycle/row instead of fp32's 4 (with automatic fast-weight-load).  Each core's kernel is a pure
LDWEIGHTS->MATMUL stream: 977 chunk matmuls accumulating X_c^T X_c
into one PSUM bank, with 1 MiB input DMAs double-buffered across the
two HWDGE queues (sync + scalar).  The device returns its partial
G_i [128,128] f32; the host sums them and does all the tiny
W-side math (trace, h(W) power series, L1) in float64.
"""

import numpy as np
from concurrent.futures import ThreadPoolExecutor

from ml_dtypes import float8_e4m3

import concourse.bacc as bacc
import concourse.mybir as mybir
from concourse import tile
from concourse.bass_utils import run_bass_kernel_spmd

D = 128
T_TRUE = 1_000_000
N_CORES = 8
CHUNKS_PER_CORE = 978            # even (DoubleRow pairs); 1472 zero-pad rows
ROWS_PER_CORE = CHUNKS_PER_CORE * D   # 125184
TILE_CHUNKS = 64                 # 64 chunks = [128, 64*128] fp8 = 1 MiB per DMA
HEAD_CHUNKS = 16                 # head ramp [16,16,32,32] so PE starts early

LAMBDA1 = 0.01
ALPHA_LAG = 0.5
RHO = 1.0
N_TERMS = 10
F32 = mybir.dt.float32
F8 = mybir.dt.float8e4
DR = mybir.MatmulPerfMode.DoubleRow


def _tile_plan(chunks, tile_chunks, head_chunks):
    """Split `chunks` into ramped head tiles, then full tiles, then a tail.

    The head ramp puts small DMAs on both HWDGE queues first so the PE
    stream starts ~2us after launch and never starves while the first
    full-size (1 MiB) DMAs are still in flight.  All tile sizes must stay
    even so each tile is a whole number of DoubleRow chunk pairs.
    """
    ramp = []
    if head_chunks and chunks > 4 * tile_chunks:
        ramp = [head_chunks // 2, head_chunks // 2, head_chunks, head_chunks,
                2 * head_chunks, 2 * head_chunks]
    rem = chunks - sum(ramp)
    plan = ramp + [tile_chunks] * (rem // tile_chunks)
    if rem % tile_chunks:
        plan.append(rem % tile_chunks)
    assert all(q % 2 == 0 for q in plan) and sum(plan) == chunks
    return plan


def _build(chunks_per_core=CHUNKS_PER_CORE, tile_chunks=TILE_CHUNKS,
           head_chunks=HEAD_CHUNKS, xbufs=8):
    rows_per_core = chunks_per_core * D
    plan = _tile_plan(chunks_per_core, tile_chunks, head_chunks)
    nc = bacc.Bacc("TRN2", target_bir_lowering=False, debug=False)
    xs = nc.dram_tensor("xs", [rows_per_core, D], F8, kind="ExternalInput")
    g = nc.dram_tensor("g", [D, D], F32, kind="ExternalOutput")

    with tile.TileContext(nc) as tc:
        with (
            tc.tile_pool(name="xpool", bufs=xbufs) as xpool,
            tc.tile_pool(name="opool", bufs=1) as opool,
            tc.tile_pool(name="gpsum", bufs=1, space="PSUM") as gpsum_pool,
        ):
            # Gram accumulation is invariant to row ordering, so partition p
            # of tile t holds the CONTIGUOUS rows base + p*q .. +q, giving
            # one contiguous descriptor per partition per DMA.
            # Two PSUM accumulators, alternated between pairs: back-to-back
            # accumulation into one bank serializes the PE slightly
            # (HW-measured ~51 vs ~48 ns/chunk with alternation).
            g_a = gpsum_pool.tile([D, D], F32, tag="ga", bufs=1)
            g_b = gpsum_pool.tile([D, D], F32, tag="gb", bufs=1)
            gs = [g_a, g_b]
            queues = [nc.sync, nc.scalar]
            n_pairs = chunks_per_core // 2
            pair = 0
            base = 0
            for t, q in enumerate(plan):
                v = (
                    xs.ap()[base : base + q * D, :]
                    .rearrange("(p q) d -> p q d", p=D, q=q)
                )
                base += q * D
                xt = xpool.tile([D, q, D], F8)
                queues[t % 2].dma_start(xt[:], v)
                # DoubleRow: one MM per chunk PAIR — contraction 256 via the
                # [128, 2, 128] k-tile AP; computes
                #   xt[:,j].T @ xt[:,j] + xt[:,j+1].T @ xt[:,j+1]
                # at ~97ns/pair vs ~160ns for two normal-mode MMs (HW-measured).
                for j in range(0, q, 2):
                    nc.tensor.matmul(
                        gs[pair % 2][:], xt[:, j : j + 2, :], xt[:, j : j + 2, :],
                        perf_mode=DR,
                        start=(pair < 2), stop=(pair >= n_pairs - 2),
                    )
                    pair += 1

            g_half = opool.tile([D, D], F32)
            nc.vector.tensor_copy(g_half[:], g_a[:])
            g_sb = opool.tile([D, D], F32)
            nc.vector.tensor_add(g_sb[:], g_half[:], g_b[:])
            nc.sync.dma_start(g.ap(), g_sb[:])

    nc.compile()
    return nc


_NC = None


def _get_nc():
    global _NC
    if _NC is None:
        _NC = _build()
    return _NC


def _quantize_shard(X, i):
    lo, hi = i * ROWS_PER_CORE, min((i + 1) * ROWS_PER_CORE, T_TRUE)
    q = np.empty((ROWS_PER_CORE, D), dtype=float8_e4m3)
    np.copyto(q[: hi - lo], X[lo:hi], casting="unsafe")
    if hi - lo < ROWS_PER_CORE:
        q[hi - lo :] = np.float32(0.0)
    return q


def _shard_inputs(X, W):
    X = np.asarray(X)
    with ThreadPoolExecutor(max_workers=N_CORES) as ex:
        shards = list(ex.map(lambda i: _quantize_shard(X, i), range(N_CORES)))
    return [{"xs": s} for s in shards], None


def _finalize(G, W):
    """All the tiny W-side math, in float64 on the host."""
    W = np.asarray(W, dtype=np.float64)
    d = W.shape[0]
    Wm = W * (1.0 - np.eye(d))
    A = np.eye(d) - Wm.T
    loss = 0.5 * np.trace(A.T @ G @ A) / T_TRUE
    WW = Wm * Wm
    total, power, factorial = 0.0, WW.copy(), 1.0
    for k in range(1, min(N_TERMS, d)):
        factorial *= k
        total += np.trace(power) / factorial
        if k < N_TERMS - 1:
            power = power @ WW
    l1 = LAMBDA1 * np.abs(Wm).sum()
    return loss + ALPHA_LAG * total + 0.5 * RHO * total * total + l1


def kernel(X, W):
    nc = _get_nc()
    in_maps, _ = _shard_inputs(X, W)
    res = run_bass_kernel_spmd(nc, in_maps, core_ids=list(range(N_CORES)))
    G = np.zeros((D, D), dtype=np.float64)
    for r in res.results:
        G += r["g"].astype(np.float64)
    return np.float32(_finalize(G, W))
